# revision 21
# baseline (speedup 1.0000x reference)
"""Butterfly rotation (10 stages, DIM=1024) on 8 Trainium2 NeuronCores.

Math: the 10-stage butterfly is linear.  Stages 0..8 mix within 512-wide
halves; stages 7/8/9 are, per dim-within-chunk p, rotations between whole
128-wide chunks with per-p angles.  Engine roofs per core (measured):
DMA ~96 us for the fp16 16 MiB in + 16 MiB out round trip (load-only is
~37 us, so the limit is bidirectional); PE fp16 matmul is 1 row/cycle at
2.4 GHz (213 ns per [128,128,512] MM); DVE stt [128,1024] fp16 ~815 ns;
ACT PSUM->SBUF copy ~731 ns.

Scheme (keeps every engine under the ~96 us DMA roofline):
  - Output chunks 0..3 ("path B"): PE applies stages 0..7 only (each
    output chunk depends on one 256-wide block = 2 input chunks), with
    kappa = cos(th8)*cos(th9) folded into the weight rows.  Stage 8 is
    then 2 DVE stt per chunk pair using coefficients t8*c9A/c9B.
  - Output chunks 4..7 ("path A"): PE applies stages 0..8 (4 input
    chunks), with cos(th9) folded in.
  - Stage 9 for all pairs (cg, cg+4) is 2 DVE stt: y_lo = q'lo - t9*q'hi,
    y_hi = t9*q'lo + q'hi, where q' = c9*q comes out of PE/stage-8 with
    the cos pre-folded.  The apparent 1/cos blowup cancels exactly: every
    stored term carries the same cos factor its coefficient divides by.
  PE/group: 4*2*2 + 4*4*2 = 48 MM (vs 80 dense) -> ~82 us; DVE: 12 stt
  -> ~78 us; ACT: 8 evictions -> ~47 us; all under DMA ~96 us.

Device layout (per core, 8192 rows; pure data parallelism): host packs
dim-major fp16: xin[g, p, c*1024 + r] = x[g*1024 + r, c*128 + p] (g: 8
row-groups of 1024 rows, c: 8 dim-chunks of 128, p: dim-within-chunk).
Output uses the same layout (slot = chunk).  Host inverse-permutes and
upcasts the fp16 output.
"""

import os
import sys

sys.path.insert(0, "/opt/trn_rl_repo")

# run_bass_kernel_spmd would try to import the (absent) axon NTFF hook if
# BASS_TRACE is set in the environment.
os.environ["BASS_NEVER_TRACE"] = "1"

import numpy as np

DIM = 1024
STAGES = 10
N_CORES = 8
ROWS_PER_CORE = 8192
GROUP_ROWS = 1024
N_GROUPS = ROWS_PER_CORE // GROUP_ROWS  # 8

# stage-8 DVE pairs (path B): chunk pairs (A, A+2) with their theta8 slice
# split "4B": chunks 0-3 get PE stages 0..7 + DVE stage 8; chunks 4-7 get
# PE stages 0..8.  "6B": chunks 0-3,4,6 on the shallow-PE path.
SPLITS = {
    "4B": [(0, 2, 0), (1, 3, 128)],  # (A, B, th8 offset)
    "6B": [(0, 2, 0), (1, 3, 128), (4, 6, 256)],
}


def _split_info(split):
    """Block index map for the wb table: B-chunks contribute 2 lhsT blocks
    (stages 0..7, 256-wide), A-chunks 4 blocks (stages 0..8, 512-wide)."""
    pairs = SPLITS[split]
    b_set = sorted(c for p in pairs for c in p[:2])
    a_set = [c for c in range(8) if c not in b_set]
    blocks = {}  # chunk -> (start index, [input chunks])
    i = 0
    for c in b_set:
        blocks[c] = (i, [2 * (c // 2) + t for t in range(2)])
        i += 2
    for c in a_set:
        blocks[c] = (i, [4 * (c // 4) + t for t in range(4)])
        i += 4
    return pairs, b_set, a_set, blocks, i  # i = total block count


def _stage_idx(dim, stage):
    stride = 2**stage
    idx_i = np.arange(dim).reshape(-1, 2 * stride)[:, :stride].ravel()
    idx_j = idx_i + stride
    return idx_i, idx_j


def _butterfly_apply(v, angles, stages):
    """Apply butterfly stages to rows of v (float64, in place) and return v."""
    for s in stages:
        idx_i, idx_j = _stage_idx(v.shape[1], s)
        c = np.cos(angles[s].astype(np.float64))
        sn = np.sin(angles[s].astype(np.float64))
        vi = v[:, idx_i].copy()
        vj = v[:, idx_j].copy()
        v[:, idx_i] = c * vi - sn * vj
        v[:, idx_j] = sn * vi + c * vj
    return v


def _host_tables(angles, split="4B"):
    """wb[k, i, m] fp16 lhsT blocks, indexed per _split_info: B-chunks get
    the stage-0..7 composite rows scaled by c8(pair)*c9[c%4], A-chunks the
    stage-0..8 composite rows scaled by c9[c%4].
    trig[p, j] f32: j=0..3 t9[cg]; 4..7 -t9[cg]; then per stage-8 pair j:
    8+2j coefA = -t8*c9A/c9B, 9+2j coefB = t8*c9B/c9A.
    """
    th = angles.astype(np.float64)
    pairs, b_set, a_set, blocks, nblk = _split_info(split)
    # _butterfly_apply on eye gives mb[i, j] = M[j, i] (M maps in->out),
    # so lhsT[k, m] = M[c*128+m, ci*128+k] = mb[ci*128+k, c*128+m].
    mb7 = _butterfly_apply(np.eye(DIM, dtype=np.float64), angles, range(8))
    mb8 = _butterfly_apply(np.eye(DIM, dtype=np.float64), angles, range(9))

    c9 = [np.cos(th[9][cg * 128 : (cg + 1) * 128]) for cg in range(4)]
    s9 = [np.sin(th[9][cg * 128 : (cg + 1) * 128]) for cg in range(4)]
    c8 = {c: np.cos(th[8][off : off + 128]) for (A, B, off) in pairs for c in (A, B)}
    s8 = {c: np.sin(th[8][off : off + 128]) for (A, B, off) in pairs for c in (A, B)}

    wb = np.empty((128, nblk, 128), dtype=np.float16)
    for c in range(8):
        kap = (c8[c] if c in c8 else 1.0) * c9[c % 4]
        start, cis = blocks[c]
        mb = mb7 if c in b_set else mb8
        for t, ci in enumerate(cis):
            blk = mb[ci * 128 : (ci + 1) * 128, c * 128 : (c + 1) * 128]
            wb[:, start + t, :] = (blk * kap[None, :]).astype(np.float16)

    trig = np.empty((128, 8 + 2 * len(pairs)), dtype=np.float32)
    for cg in range(4):
        t9 = s9[cg] / c9[cg]
        trig[:, cg] = t9
        trig[:, 4 + cg] = -t9
    for j, (A, B, off) in enumerate(pairs):
        t8 = s8[A] / c8[A]
        trig[:, 8 + 2 * j] = -t8 * c9[A % 4] / c9[B % 4]  # coefA
        trig[:, 9 + 2 * j] = t8 * c9[B % 4] / c9[A % 4]  # coefB
    return wb, trig


def _pack_x(x_core, n_groups=N_GROUPS):
    # [G*1024, 1024] -> [G, 128, 8192] with xin[g, p, c*1024+r] = x[g*1024+r, c*128+p]
    g = x_core.reshape(n_groups, GROUP_ROWS, 8, 128)
    return np.ascontiguousarray(
        g.transpose(0, 3, 2, 1).reshape(n_groups, 128, 8 * GROUP_ROWS)
    )


def _unpack_y(y_packed, n_groups=N_GROUPS):
    # yout[g, p, c*1024 + r] = y[g*1024 + r, c*128 + p]  (slot = chunk)
    g = y_packed.reshape(n_groups, 128, 8, GROUP_ROWS)
    g = g.transpose(0, 3, 2, 1)  # [g, r, c, p]
    return np.ascontiguousarray(g.reshape(n_groups * GROUP_ROWS, DIM))


def _patch_tile_drain():
    """Workaround: this walrus build cannot encode semaphore waits on a
    sequencer Drain/NoOp with >1 wait ("Too many sync wait commands").
    Re-emit the TileContext tail waits as one nop per semaphore."""
    from concourse import mybir, tile
    from concourse.vector_clock import ScopedClock

    if getattr(tile.TileContext, "_drain_patched", False):
        return

    def _drain_and_barrier(self, tick_clock, wait_clock):
        nop_inst = self.nc.sync.nop(nofuse=True)
        wait_clock.add_sem_waits(
            nop_inst.ins, ScopedClock({None: tick_clock.global_clock})
        )
        si = nop_inst.ins.sync_info
        if si is not None and si.on_wait and len(si.on_wait) > 1:
            extra = si.on_wait[1:]
            si.on_wait = si.on_wait[:1]
            for w in extra:
                extra_nop = self.nc.sync.nop(nofuse=True)
                esi = extra_nop.ins.sync_info
                if esi is None:
                    extra_nop.ins.sync_info = mybir.SyncInfo(on_wait=[w], on_update=[])
                else:
                    esi.on_wait = list(esi.on_wait or []) + [w]
        self.nc.sync.drain()
        self.nc.all_engine_barrier()
        assert self.sems is not None
        popped = self.nc._tile_sem_poison_stack.pop()
        assert popped is self._sem_poison
        self.nc.clear_and_free_semaphores(list(self.sems.allocated().values()))
        self.nc.all_engine_barrier()

    tile.TileContext._drain_and_barrier = _drain_and_barrier
    tile.TileContext._drain_patched = True


def _split_multi_waits(nc, limit=1):
    """This walrus build encodes at most `limit` semaphore wait(s) per
    instruction ("Too many sync wait commands").  Hoist excess waits onto
    same-engine NoOps inserted immediately before the instruction."""
    from concourse import mybir

    counter = [0]

    def fresh_nop(engine, waits):
        counter[0] += 1
        nop = mybir.InstNoOp(
            name=f"waitsplit-{counter[0]}",
            engine=engine,
            ins=[],
            outs=[],
            bass_nofuse=True,
            sync_info=mybir.SyncInfo(on_wait=list(waits), on_update=[]),
        )
        nc.register_instruction(nop, overwrite=True)
        return nop

    for fn in nc.m.functions:
        for bb in fn.blocks:
            changed = False
            new = []
            for inst in bb.instructions:
                si = getattr(inst, "sync_info", None)
                if si is not None and si.on_wait and len(si.on_wait) > limit:
                    extra = si.on_wait[: len(si.on_wait) - limit]
                    si.on_wait = si.on_wait[len(si.on_wait) - limit :]
                    for k in range(0, len(extra), limit):
                        new.append(fresh_nop(inst.engine, extra[k : k + limit]))
                    changed = True
                new.append(inst)
            if changed:
                bb.instructions = new
    return nc


def build_bass_std(n_groups=N_GROUPS, reps=1, upto="full", load_split=4,
                   store_rings="swdge", pair_psum=True, store_grain=2048,
                   split="4B"):
    """Build the Bass module for one core processing n_groups row-groups.
    reps>1 repeats the whole pipeline in-NEFF (for timing calibration).
    upto: 'pe' | 'evict' | 'dve' | 'full' truncates the pipeline (for
    engine-attribution benchmarks).  load_split: number of load DMAs per
    group.  store_rings: 'act' | 'split' (alternate SP/ACT).  pair_psum:
    evict stage-8/9 chunk pairs as one [128,2048] ACT op."""
    _patch_tile_drain()
    from concourse import bass, mybir, tile

    stage_n = ["pe", "evict", "dve", "full"].index(upto)
    f16 = mybir.dt.float16
    f32 = mybir.dt.float32
    nc = bass.Bass("TRN2", target_bir_lowering=False, debug=False)
    xin = nc.dram_tensor("xin", [n_groups, 128, 8192], f16, kind="ExternalInput")
    pairs, b_set, a_set, blocks, nblk = _split_info(split)
    ntg = 8 + 2 * len(pairs)
    wbd = nc.dram_tensor("wb", [128, nblk, 128], f16, kind="ExternalInput")
    trig = nc.dram_tensor("trig", [128, ntg], f32, kind="ExternalInput")
    yout = nc.dram_tensor("yout", [n_groups, 128, 8192], f16, kind="ExternalOutput")

    mult = mybir.AluOpType.mult
    add = mybir.AluOpType.add

    # PE emission order: path-B stage-8 partners adjacent, then path-A in
    # the order the stage-9 waves consume them.
    PE_ORDER = [0, 2, 1, 3, 4, 6, 5, 7]

    def mm_chunk(psum, wtile, c, xt):
        """Accumulate output chunk c's [128, 1024] PSUM tile."""
        start, cis = blocks[c]
        idxs = [start + t for t in range(len(cis))]
        for h in range(2):
            for j, (i, ci) in enumerate(zip(idxs, cis)):
                nc.tensor.matmul(
                    psum[:, h * 512 : (h + 1) * 512],
                    wtile[:, i, :],
                    xt[:, ci * 1024 + h * 512 : ci * 1024 + (h + 1) * 512],
                    start=(j == 0),
                    stop=(j == len(idxs) - 1),
                )

    with tile.TileContext(nc) as tc:
        with (
            tc.tile_pool(name="wp", bufs=1) as wp,
            tc.tile_pool(name="xp", bufs=3) as xp,
            tc.tile_pool(name="yp", bufs=2) as yp,
            tc.tile_pool(name="ep", bufs=12) as ep,
            tc.tile_pool(name="qp", bufs=6) as qp,
            tc.tile_pool(name="ps", bufs=2 if pair_psum else 4, space="PSUM") as psp,
        ):
            wb = wp.tile([128, nblk, 128], f16)
            nc.sync.dma_start(wb[:], wbd.ap()[:])
            tg = wp.tile([128, ntg], f32)
            nc.sync.dma_start(tg[:], trig.ap()[:])

            for g in [g for _ in range(reps) for g in range(n_groups)]:
                xt = xp.tile([128, 8192], f16)
                lw = 8192 // load_split
                for ls in range(load_split):
                    nc.sync.dma_start(
                        xt[:, ls * lw : (ls + 1) * lw],
                        xin.ap()[g][:, ls * lw : (ls + 1) * lw],
                    )
                yt = yp.tile([128, 8192], f16)

                # PE + ACT eviction per chunk (or per stage-8/9 pair)
                w = {}
                if pair_psum:
                    for ca, cb in ((0, 2), (1, 3), (4, 6), (5, 7)):
                        p = psp.tile([128, 2048], f32, tag="ps")
                        mm_chunk(p[:, 0:1024], wb, ca, xt)
                        mm_chunk(p[:, 1024:2048], wb, cb, xt)
                        if stage_n >= 1:
                            e = ep.tile(
                                [128, 2048], f16, tag="e", name=f"w{ca}{cb}"
                            )
                            nc.scalar.copy(e[:], p[:])
                            w[ca], w[cb] = e[:, 0:1024], e[:, 1024:2048]
                else:
                    for c in PE_ORDER:
                        p = psp.tile([128, 1024], f32, tag="ps")
                        mm_chunk(p, wb, c, xt)
                        if stage_n >= 1:
                            e = ep.tile([128, 1024], f16, tag="e", name=f"w{c}")
                            nc.scalar.copy(e[:], p[:])
                            w[c] = e[:]
                if stage_n < 2:
                    continue

                # DVE stage 8 (path-B pairs): q'A = coefA*wB + wA, etc.
                q = {}
                for j, (A, B, _) in enumerate(pairs):
                    qA = qp.tile([128, 1024], f16, tag="q", name=f"q{A}")
                    nc.vector.scalar_tensor_tensor(
                        qA[:], w[B], tg[:, 8 + 2 * j : 9 + 2 * j], w[A],
                        mult, add,
                    )
                    qB = qp.tile([128, 1024], f16, tag="q", name=f"q{B}")
                    nc.vector.scalar_tensor_tensor(
                        qB[:], w[A], tg[:, 9 + 2 * j : 10 + 2 * j], w[B],
                        mult, add,
                    )
                    q[A], q[B] = qA[:], qB[:]
                for c in a_set:
                    q[c] = w[c]

                # DVE stage 9: pairs (cg, cg+4) into the output tile.  All
                # lo outputs (slots 0..3) first so their stores fire while
                # the hi waves still run.
                def store(sl):
                    # stores ride SWDGE (Pool engine, otherwise idle): the
                    # issuing engine pays ~1.5-2 ns/KB of descriptor-gen, so
                    # putting stores on ACT (which also evicts) serializes
                    # against the evictions
                    eng = {"act": nc.scalar, "swdge": nc.gpsimd}[store_rings]
                    eng.dma_start(
                        yout.ap()[g][:, sl : sl + store_grain],
                        yt[:, sl : sl + store_grain],
                    )

                for cg in range(4):
                    nc.vector.scalar_tensor_tensor(
                        yt[:, cg * 1024 : (cg + 1) * 1024],
                        q[cg + 4], tg[:, 4 + cg : 5 + cg], q[cg],
                        mult, add,
                    )
                    if stage_n >= 3 and (cg + 1) * 1024 % store_grain == 0:
                        store((cg + 1) * 1024 - store_grain)
                for cg in range(4):
                    nc.vector.scalar_tensor_tensor(
                        yt[:, (cg + 4) * 1024 : (cg + 5) * 1024],
                        q[cg], tg[:, cg : cg + 1], q[cg + 4],
                        mult, add,
                    )
                    if stage_n >= 3 and (cg + 1) * 1024 % store_grain == 0:
                        store((cg + 5) * 1024 - store_grain)
    _split_multi_waits(nc)
    return nc


def build_bass(n_groups=N_GROUPS, reps=1, upto="full", split="4B"):
    """Supergroup variant: 2 row-groups per pipeline unit so every DVE stt
    is 2048 wide (halves DVE per-instruction fixed costs).  Loads on SP,
    evictions pair-granular on ACT, stores quarter-granular on SWDGE."""
    _patch_tile_drain()
    from concourse import bass, mybir, tile

    assert n_groups % 2 == 0
    stage_n = ["pe", "evict", "dve", "full"].index(upto)
    f16 = mybir.dt.float16
    f32 = mybir.dt.float32
    nc = bass.Bass("TRN2", target_bir_lowering=False, debug=False)
    xin = nc.dram_tensor("xin", [n_groups, 128, 8192], f16, kind="ExternalInput")
    pairs, b_set, a_set, blocks, nblk = _split_info(split)
    ntg = 8 + 2 * len(pairs)
    wbd = nc.dram_tensor("wb", [128, nblk, 128], f16, kind="ExternalInput")
    trig = nc.dram_tensor("trig", [128, ntg], f32, kind="ExternalInput")
    yout = nc.dram_tensor("yout", [n_groups, 128, 8192], f16, kind="ExternalOutput")

    mult = mybir.AluOpType.mult
    add = mybir.AluOpType.add

    def mm_chunk(psum, wtile, c, xg):
        start, cis = blocks[c]
        idxs = [start + t for t in range(len(cis))]
        for h in range(2):
            for j, (i, ci) in enumerate(zip(idxs, cis)):
                nc.tensor.matmul(
                    psum[:, h * 512 : (h + 1) * 512],
                    wtile[:, i, :],
                    xg[:, ci * 1024 + h * 512 : ci * 1024 + (h + 1) * 512],
                    start=(j == 0),
                    stop=(j == len(idxs) - 1),
                )

    with tile.TileContext(nc) as tc:
        with (
            tc.tile_pool(name="wp", bufs=1) as wp,
            tc.tile_pool(name="xp", bufs=2) as xp,
            tc.tile_pool(name="yp", bufs=2) as yp,
            tc.tile_pool(name="ep", bufs=5) as ep,
            tc.tile_pool(name="qp", bufs=4) as qp,
            tc.tile_pool(name="ps", bufs=2, space="PSUM") as psp,
        ):
            wb = wp.tile([128, nblk, 128], f16)
            nc.sync.dma_start(wb[:], wbd.ap()[:])
            tg = wp.tile([128, ntg], f32)
            nc.sync.dma_start(tg[:], trig.ap()[:])

            for sg in [s for _ in range(reps) for s in range(n_groups // 2)]:
                g0 = 2 * sg
                xt = xp.tile([128, 2, 8192], f16)
                for gg in range(2):
                    for ls in range(4):
                        nc.sync.dma_start(
                            xt[:, gg, ls * 2048 : (ls + 1) * 2048],
                            xin.ap()[g0 + gg][:, ls * 2048 : (ls + 1) * 2048],
                        )
                yt = yp.tile([128, 2, 8192], f16)

                # PE + pair-granular ACT eviction, per (pair, group)
                w = {}
                for ca, cb in ((0, 2), (1, 3), (4, 6), (5, 7)):
                    e = ep.tile([128, 2, 2048], f16, tag="e", name=f"w{ca}{cb}")
                    for gg in range(2):
                        p = psp.tile([128, 2048], f32, tag="ps")
                        mm_chunk(p[:, 0:1024], wb, ca, xt[:, gg, :])
                        mm_chunk(p[:, 1024:2048], wb, cb, xt[:, gg, :])
                        if stage_n >= 1:
                            nc.scalar.copy(e[:, gg, :], p[:])
                    w[ca] = e[:, :, 0:1024]
                    w[cb] = e[:, :, 1024:2048]
                if stage_n < 2:
                    continue

                # DVE stage 8 (2048-wide, strided): q'A = coefA*wB + wA
                q = {}
                for j, (A, B, _) in enumerate(pairs):
                    qA = qp.tile([128, 2, 1024], f16, tag="q", name=f"q{A}")
                    nc.vector.scalar_tensor_tensor(
                        qA[:], w[B], tg[:, 8 + 2 * j : 9 + 2 * j], w[A],
                        mult, add,
                    )
                    qB = qp.tile([128, 2, 1024], f16, tag="q", name=f"q{B}")
                    nc.vector.scalar_tensor_tensor(
                        qB[:], w[A], tg[:, 9 + 2 * j : 10 + 2 * j], w[B],
                        mult, add,
                    )
                    q[A], q[B] = qA[:], qB[:]
                for c in a_set:
                    q[c] = w[c]

                def store(gg, sl):
                    nc.gpsimd.dma_start(
                        yout.ap()[g0 + gg][:, sl : sl + 2048],
                        yt[:, gg, sl : sl + 2048],
                    )

                # DVE stage 9 (2048-wide): lo slots first, then hi
                for cg in range(4):
                    nc.vector.scalar_tensor_tensor(
                        yt[:, :, cg * 1024 : (cg + 1) * 1024],
                        q[cg + 4], tg[:, 4 + cg : 5 + cg], q[cg],
                        mult, add,
                    )
                    if stage_n >= 3 and cg % 2 == 1:
                        for gg in range(2):
                            store(gg, (cg - 1) * 1024)
                for cg in range(4):
                    nc.vector.scalar_tensor_tensor(
                        yt[:, :, (cg + 4) * 1024 : (cg + 5) * 1024],
                        q[cg], tg[:, cg : cg + 1], q[cg + 4],
                        mult, add,
                    )
                    if stage_n >= 3 and cg % 2 == 1:
                        for gg in range(2):
                            store(gg, (cg + 3) * 1024)
    _split_multi_waits(nc)
    return nc


_CACHE = {}

# selected configuration (see ab.py benchmarks)
CONFIG_SPLIT = "4B"
CONFIG_BUILDER = "sg2"  # sg2 == the default build_bass


def _get_nc(n_groups=N_GROUPS):
    if n_groups not in _CACHE:
        if CONFIG_BUILDER == "sg2":
            _CACHE[n_groups] = build_bass(n_groups, split=CONFIG_SPLIT)
        else:
            _CACHE[n_groups] = build_bass_std(n_groups, split=CONFIG_SPLIT)
    return _CACHE[n_groups]


def make_in_maps(x, angles, split=None):
    """Pack full inputs into per-core in_maps (list of dicts)."""
    x = np.asarray(x, dtype=np.float32)
    angles = np.asarray(angles, dtype=np.float32)
    wb, trig = _host_tables(angles, split or CONFIG_SPLIT)
    flat = x.reshape(-1, DIM).astype(np.float16)
    in_maps = []
    for k in range(N_CORES):
        shard = flat[k * ROWS_PER_CORE : (k + 1) * ROWS_PER_CORE]
        in_maps.append({"xin": _pack_x(shard), "wb": wb, "trig": trig})
    return in_maps


def kernel(x, angles):
    from concourse.bass_utils import run_bass_kernel_spmd

    x = np.asarray(x)
    orig_shape = x.shape
    in_maps = make_in_maps(x, angles)
    nc = _get_nc()
    res = run_bass_kernel_spmd(nc, in_maps, core_ids=list(range(N_CORES)))
    parts = [_unpack_y(res.results[k]["yout"]) for k in range(N_CORES)]
    out = np.concatenate(parts, axis=0).reshape(orig_shape)
    return out.astype(np.float32)


# revision 23
# speedup vs baseline: 1.7998x; 1.7998x over previous
"""Butterfly rotation (10 stages, DIM=1024) on 8 Trainium2 NeuronCores.

Math: the 10-stage butterfly is linear.  Stages 0..8 mix within 512-wide
halves; stages 8 and 9 are, per dim-within-chunk p, rotations between
whole 128-wide chunks with per-p angles.  Per-core engine roofs
(measured): DMA round trip (fp16 16 MiB in + 16 MiB out) ~64-86 us
bidirectional; PE fp16 matmul ~0.4 ns/row ([128,128,512] MM ~200 ns);
DVE stt [128,2048] fp16 ~1.35 us; ACT PSUM->SBUF copy [128,2048] ~1.5 us.

Scheme ("4B" split, supergroups of 2 row-groups):
  - Output chunks 0..3 ("path B"): PE applies stages 0..7 only (one
    256-wide block = 2 input chunks each), with cos(th8)*cos(th9) folded
    into the weight rows.  Stage 8 is then 2 DVE stt per chunk pair
    (0,2)/(1,3) with coefficients -+t8*c9A/c9B.
  - Output chunks 4..7 ("path A"): PE applies stages 0..8 (4 input
    chunks), with cos(th9) folded in.
  - Stage 9 for pairs (cg, cg+4) is 2 DVE stt: y_lo = q'lo - t9*q'hi,
    y_hi = t9*q'lo + q'hi, where q' = c9*q arrives from PE/stage-8 with
    the cos pre-folded.  The apparent 1/cos blowup cancels exactly: every
    stored term carries the same cos factor its coefficient divides by
    (verified: rel err ~8e-4 incl. max|t9| ~ 1.4e3 draws).
  Engine budget per core: PE 48 MM/group (vs 80 for the dense baseline),
  DVE 12 stt per supergroup at 2048 wide, ACT 8 pair evictions per
  supergroup, loads on the SP HWDGE ring, stores on SWDGE (Pool) so no
  engine both computes and triggers DMA.  Measured ~64-67 us vs the
  142.8 us dense-PE baseline.

Pipeline per supergroup (2 row-groups, 2048 rows): 8 quarter loads ->
PE per (stage-8/9 pair, group) into [128,2048] PSUM -> ACT pair
eviction to fp16 SBUF -> DVE stage-8 (2048-wide strided stt) -> DVE
stage-9 into the output tile (lo slots first) -> 8 quarter stores fired
as their slots complete.

Device layout (per core, 8192 rows; pure data parallelism): host packs
dim-major fp16: xin[g, p, c*1024 + r] = x[g*1024 + r, c*128 + p] (g: 8
row-groups of 1024 rows, c: 8 dim-chunks of 128, p: dim-within-chunk).
Output uses the same layout (slot = chunk).  Host inverse-permutes and
upcasts the fp16 output.
"""

import os
import sys

sys.path.insert(0, "/opt/trn_rl_repo")

# run_bass_kernel_spmd would try to import the (absent) axon NTFF hook if
# BASS_TRACE is set in the environment.
os.environ["BASS_NEVER_TRACE"] = "1"

import numpy as np

DIM = 1024
STAGES = 10
N_CORES = 8
ROWS_PER_CORE = 8192
GROUP_ROWS = 1024
N_GROUPS = ROWS_PER_CORE // GROUP_ROWS  # 8

# stage-8 DVE pairs (path B): chunk pairs (A, A+2) with their theta8 slice
# split "4B": chunks 0-3 get PE stages 0..7 + DVE stage 8; chunks 4-7 get
# PE stages 0..8.  "6B": chunks 0-3,4,6 on the shallow-PE path.
SPLITS = {
    "2B": [(0, 2, 0)],  # (A, B, th8 offset)
    "4B": [(0, 2, 0), (1, 3, 128)],
    "6B": [(0, 2, 0), (1, 3, 128), (4, 6, 256)],
}


def _split_info(split):
    """Block index map for the wb table: B-chunks contribute 2 lhsT blocks
    (stages 0..7, 256-wide), A-chunks 4 blocks (stages 0..8, 512-wide)."""
    pairs = SPLITS[split]
    b_set = sorted(c for p in pairs for c in p[:2])
    a_set = [c for c in range(8) if c not in b_set]
    blocks = {}  # chunk -> (start index, [input chunks])
    i = 0
    for c in b_set:
        blocks[c] = (i, [2 * (c // 2) + t for t in range(2)])
        i += 2
    for c in a_set:
        blocks[c] = (i, [4 * (c // 4) + t for t in range(4)])
        i += 4
    return pairs, b_set, a_set, blocks, i  # i = total block count


def _stage_idx(dim, stage):
    stride = 2**stage
    idx_i = np.arange(dim).reshape(-1, 2 * stride)[:, :stride].ravel()
    idx_j = idx_i + stride
    return idx_i, idx_j


def _butterfly_apply(v, angles, stages):
    """Apply butterfly stages to rows of v (float64, in place) and return v."""
    for s in stages:
        idx_i, idx_j = _stage_idx(v.shape[1], s)
        c = np.cos(angles[s].astype(np.float64))
        sn = np.sin(angles[s].astype(np.float64))
        vi = v[:, idx_i].copy()
        vj = v[:, idx_j].copy()
        v[:, idx_i] = c * vi - sn * vj
        v[:, idx_j] = sn * vi + c * vj
    return v


def _host_tables(angles, split="4B"):
    """wb[k, i, m] fp16 lhsT blocks, indexed per _split_info: B-chunks get
    the stage-0..7 composite rows scaled by c8(pair)*c9[c%4], A-chunks the
    stage-0..8 composite rows scaled by c9[c%4].
    trig[p, j] f32: j=0..3 t9[cg]; 4..7 -t9[cg]; then per stage-8 pair j:
    8+2j coefA = -t8*c9A/c9B, 9+2j coefB = t8*c9B/c9A.
    """
    th = angles.astype(np.float64)
    pairs, b_set, a_set, blocks, nblk = _split_info(split)
    # _butterfly_apply on eye gives mb[i, j] = M[j, i] (M maps in->out),
    # so lhsT[k, m] = M[c*128+m, ci*128+k] = mb[ci*128+k, c*128+m].
    mb7 = _butterfly_apply(np.eye(DIM, dtype=np.float64), angles, range(8))
    mb8 = _butterfly_apply(np.eye(DIM, dtype=np.float64), angles, range(9))

    c9 = [np.cos(th[9][cg * 128 : (cg + 1) * 128]) for cg in range(4)]
    s9 = [np.sin(th[9][cg * 128 : (cg + 1) * 128]) for cg in range(4)]
    c8 = {c: np.cos(th[8][off : off + 128]) for (A, B, off) in pairs for c in (A, B)}
    s8 = {c: np.sin(th[8][off : off + 128]) for (A, B, off) in pairs for c in (A, B)}

    wb = np.empty((128, nblk, 128), dtype=np.float16)
    for c in range(8):
        kap = (c8[c] if c in c8 else 1.0) * c9[c % 4]
        start, cis = blocks[c]
        mb = mb7 if c in b_set else mb8
        for t, ci in enumerate(cis):
            blk = mb[ci * 128 : (ci + 1) * 128, c * 128 : (c + 1) * 128]
            wb[:, start + t, :] = (blk * kap[None, :]).astype(np.float16)

    trig = np.empty((128, 8 + 2 * len(pairs)), dtype=np.float32)
    for cg in range(4):
        t9 = s9[cg] / c9[cg]
        trig[:, cg] = t9
        trig[:, 4 + cg] = -t9
    for j, (A, B, off) in enumerate(pairs):
        t8 = s8[A] / c8[A]
        trig[:, 8 + 2 * j] = -t8 * c9[A % 4] / c9[B % 4]  # coefA
        trig[:, 9 + 2 * j] = t8 * c9[B % 4] / c9[A % 4]  # coefB
    return wb, trig


def _pack_x(x_core, n_groups=N_GROUPS):
    # [G*1024, 1024] -> [G, 128, 8192] with xin[g, p, c*1024+r] = x[g*1024+r, c*128+p]
    g = x_core.reshape(n_groups, GROUP_ROWS, 8, 128)
    return np.ascontiguousarray(
        g.transpose(0, 3, 2, 1).reshape(n_groups, 128, 8 * GROUP_ROWS)
    )


def _unpack_y(y_packed, n_groups=N_GROUPS):
    # yout[g, p, c*1024 + r] = y[g*1024 + r, c*128 + p]  (slot = chunk)
    g = y_packed.reshape(n_groups, 128, 8, GROUP_ROWS)
    g = g.transpose(0, 3, 2, 1)  # [g, r, c, p]
    return np.ascontiguousarray(g.reshape(n_groups * GROUP_ROWS, DIM))


def _patch_tile_drain():
    """Workaround: this walrus build cannot encode semaphore waits on a
    sequencer Drain/NoOp with >1 wait ("Too many sync wait commands").
    Re-emit the TileContext tail waits as one nop per semaphore."""
    from concourse import mybir, tile
    from concourse.vector_clock import ScopedClock

    if getattr(tile.TileContext, "_drain_patched", False):
        return

    def _drain_and_barrier(self, tick_clock, wait_clock):
        nop_inst = self.nc.sync.nop(nofuse=True)
        wait_clock.add_sem_waits(
            nop_inst.ins, ScopedClock({None: tick_clock.global_clock})
        )
        si = nop_inst.ins.sync_info
        if si is not None and si.on_wait and len(si.on_wait) > 1:
            extra = si.on_wait[1:]
            si.on_wait = si.on_wait[:1]
            for w in extra:
                extra_nop = self.nc.sync.nop(nofuse=True)
                esi = extra_nop.ins.sync_info
                if esi is None:
                    extra_nop.ins.sync_info = mybir.SyncInfo(on_wait=[w], on_update=[])
                else:
                    esi.on_wait = list(esi.on_wait or []) + [w]
        self.nc.sync.drain()
        self.nc.all_engine_barrier()
        assert self.sems is not None
        popped = self.nc._tile_sem_poison_stack.pop()
        assert popped is self._sem_poison
        self.nc.clear_and_free_semaphores(list(self.sems.allocated().values()))
        self.nc.all_engine_barrier()

    tile.TileContext._drain_and_barrier = _drain_and_barrier
    tile.TileContext._drain_patched = True


def _split_multi_waits(nc, limit=1):
    """This walrus build encodes at most `limit` semaphore wait(s) per
    instruction ("Too many sync wait commands").  Hoist excess waits onto
    same-engine NoOps inserted immediately before the instruction."""
    from concourse import mybir

    counter = [0]

    def fresh_nop(engine, waits):
        counter[0] += 1
        nop = mybir.InstNoOp(
            name=f"waitsplit-{counter[0]}",
            engine=engine,
            ins=[],
            outs=[],
            bass_nofuse=True,
            sync_info=mybir.SyncInfo(on_wait=list(waits), on_update=[]),
        )
        nc.register_instruction(nop, overwrite=True)
        return nop

    for fn in nc.m.functions:
        for bb in fn.blocks:
            changed = False
            new = []
            for inst in bb.instructions:
                si = getattr(inst, "sync_info", None)
                if si is not None and si.on_wait and len(si.on_wait) > limit:
                    extra = si.on_wait[: len(si.on_wait) - limit]
                    si.on_wait = si.on_wait[len(si.on_wait) - limit :]
                    for k in range(0, len(extra), limit):
                        new.append(fresh_nop(inst.engine, extra[k : k + limit]))
                    changed = True
                new.append(inst)
            if changed:
                bb.instructions = new
    return nc


def build_bass_std(n_groups=N_GROUPS, reps=1, upto="full", load_split=4,
                   store_rings="swdge", pair_psum=True, store_grain=2048,
                   split="4B"):
    """Build the Bass module for one core processing n_groups row-groups.
    reps>1 repeats the whole pipeline in-NEFF (for timing calibration).
    upto: 'pe' | 'evict' | 'dve' | 'full' truncates the pipeline (for
    engine-attribution benchmarks).  load_split: number of load DMAs per
    group.  store_rings: 'act' | 'split' (alternate SP/ACT).  pair_psum:
    evict stage-8/9 chunk pairs as one [128,2048] ACT op."""
    _patch_tile_drain()
    from concourse import bass, mybir, tile

    stage_n = ["pe", "evict", "dve", "full"].index(upto)
    f16 = mybir.dt.float16
    f32 = mybir.dt.float32
    nc = bass.Bass("TRN2", target_bir_lowering=False, debug=False)
    xin = nc.dram_tensor("xin", [n_groups, 128, 8192], f16, kind="ExternalInput")
    pairs, b_set, a_set, blocks, nblk = _split_info(split)
    ntg = 8 + 2 * len(pairs)
    wbd = nc.dram_tensor("wb", [128, nblk, 128], f16, kind="ExternalInput")
    trig = nc.dram_tensor("trig", [128, ntg], f32, kind="ExternalInput")
    yout = nc.dram_tensor("yout", [n_groups, 128, 8192], f16, kind="ExternalOutput")

    mult = mybir.AluOpType.mult
    add = mybir.AluOpType.add

    # PE emission order: path-B stage-8 partners adjacent, then path-A in
    # the order the stage-9 waves consume them.
    PE_ORDER = [0, 2, 1, 3, 4, 6, 5, 7]

    def mm_chunk(psum, wtile, c, xt):
        """Accumulate output chunk c's [128, 1024] PSUM tile."""
        start, cis = blocks[c]
        idxs = [start + t for t in range(len(cis))]
        for h in range(2):
            for j, (i, ci) in enumerate(zip(idxs, cis)):
                nc.tensor.matmul(
                    psum[:, h * 512 : (h + 1) * 512],
                    wtile[:, i, :],
                    xt[:, ci * 1024 + h * 512 : ci * 1024 + (h + 1) * 512],
                    start=(j == 0),
                    stop=(j == len(idxs) - 1),
                )

    with tile.TileContext(nc) as tc:
        with (
            tc.tile_pool(name="wp", bufs=1) as wp,
            tc.tile_pool(name="xp", bufs=3) as xp,
            tc.tile_pool(name="yp", bufs=2) as yp,
            tc.tile_pool(name="ep", bufs=12) as ep,
            tc.tile_pool(name="qp", bufs=6) as qp,
            tc.tile_pool(name="ps", bufs=2 if pair_psum else 4, space="PSUM") as psp,
        ):
            wb = wp.tile([128, nblk, 128], f16)
            nc.sync.dma_start(wb[:], wbd.ap()[:])
            tg = wp.tile([128, ntg], f32)
            nc.sync.dma_start(tg[:], trig.ap()[:])

            for g in [g for _ in range(reps) for g in range(n_groups)]:
                xt = xp.tile([128, 8192], f16)
                lw = 8192 // load_split
                for ls in range(load_split):
                    nc.sync.dma_start(
                        xt[:, ls * lw : (ls + 1) * lw],
                        xin.ap()[g][:, ls * lw : (ls + 1) * lw],
                    )
                yt = yp.tile([128, 8192], f16)

                # PE + ACT eviction per chunk (or per stage-8/9 pair)
                w = {}
                if pair_psum:
                    for ca, cb in ((0, 2), (1, 3), (4, 6), (5, 7)):
                        p = psp.tile([128, 2048], f32, tag="ps")
                        mm_chunk(p[:, 0:1024], wb, ca, xt)
                        mm_chunk(p[:, 1024:2048], wb, cb, xt)
                        if stage_n >= 1:
                            e = ep.tile(
                                [128, 2048], f16, tag="e", name=f"w{ca}{cb}"
                            )
                            nc.scalar.copy(e[:], p[:])
                            w[ca], w[cb] = e[:, 0:1024], e[:, 1024:2048]
                else:
                    for c in PE_ORDER:
                        p = psp.tile([128, 1024], f32, tag="ps")
                        mm_chunk(p, wb, c, xt)
                        if stage_n >= 1:
                            e = ep.tile([128, 1024], f16, tag="e", name=f"w{c}")
                            nc.scalar.copy(e[:], p[:])
                            w[c] = e[:]
                if stage_n < 2:
                    continue

                # DVE stage 8 (path-B pairs): q'A = coefA*wB + wA, etc.
                q = {}
                for j, (A, B, _) in enumerate(pairs):
                    qA = qp.tile([128, 1024], f16, tag="q", name=f"q{A}")
                    nc.vector.scalar_tensor_tensor(
                        qA[:], w[B], tg[:, 8 + 2 * j : 9 + 2 * j], w[A],
                        mult, add,
                    )
                    qB = qp.tile([128, 1024], f16, tag="q", name=f"q{B}")
                    nc.vector.scalar_tensor_tensor(
                        qB[:], w[A], tg[:, 9 + 2 * j : 10 + 2 * j], w[B],
                        mult, add,
                    )
                    q[A], q[B] = qA[:], qB[:]
                for c in a_set:
                    q[c] = w[c]

                # DVE stage 9: pairs (cg, cg+4) into the output tile.  All
                # lo outputs (slots 0..3) first so their stores fire while
                # the hi waves still run.
                def store(sl):
                    # stores ride SWDGE (Pool engine, otherwise idle): the
                    # issuing engine pays ~1.5-2 ns/KB of descriptor-gen, so
                    # putting stores on ACT (which also evicts) serializes
                    # against the evictions
                    eng = {"act": nc.scalar, "swdge": nc.gpsimd}[store_rings]
                    eng.dma_start(
                        yout.ap()[g][:, sl : sl + store_grain],
                        yt[:, sl : sl + store_grain],
                    )

                for cg in range(4):
                    nc.vector.scalar_tensor_tensor(
                        yt[:, cg * 1024 : (cg + 1) * 1024],
                        q[cg + 4], tg[:, 4 + cg : 5 + cg], q[cg],
                        mult, add,
                    )
                    if stage_n >= 3 and (cg + 1) * 1024 % store_grain == 0:
                        store((cg + 1) * 1024 - store_grain)
                for cg in range(4):
                    nc.vector.scalar_tensor_tensor(
                        yt[:, (cg + 4) * 1024 : (cg + 5) * 1024],
                        q[cg], tg[:, cg : cg + 1], q[cg + 4],
                        mult, add,
                    )
                    if stage_n >= 3 and (cg + 1) * 1024 % store_grain == 0:
                        store((cg + 5) * 1024 - store_grain)
    _split_multi_waits(nc)
    return nc


def build_bass(n_groups=N_GROUPS, reps=1, upto="full", split="4B",
               ep_bufs=5, qp_bufs=4):
    """Supergroup variant: 2 row-groups per pipeline unit so every DVE stt
    is 2048 wide (halves DVE per-instruction fixed costs).  Loads on SP,
    evictions pair-granular on ACT, stores quarter-granular on SWDGE."""
    _patch_tile_drain()
    from concourse import bass, mybir, tile

    assert n_groups % 2 == 0
    stage_n = ["pe", "evict", "dve", "full"].index(upto)
    f16 = mybir.dt.float16
    f32 = mybir.dt.float32
    nc = bass.Bass("TRN2", target_bir_lowering=False, debug=False)
    xin = nc.dram_tensor("xin", [n_groups, 128, 8192], f16, kind="ExternalInput")
    pairs, b_set, a_set, blocks, nblk = _split_info(split)
    ntg = 8 + 2 * len(pairs)
    wbd = nc.dram_tensor("wb", [128, nblk, 128], f16, kind="ExternalInput")
    trig = nc.dram_tensor("trig", [128, ntg], f32, kind="ExternalInput")
    yout = nc.dram_tensor("yout", [n_groups, 128, 8192], f16, kind="ExternalOutput")

    mult = mybir.AluOpType.mult
    add = mybir.AluOpType.add

    def mm_chunk(psum, wtile, c, xg):
        start, cis = blocks[c]
        idxs = [start + t for t in range(len(cis))]
        for h in range(2):
            for j, (i, ci) in enumerate(zip(idxs, cis)):
                nc.tensor.matmul(
                    psum[:, h * 512 : (h + 1) * 512],
                    wtile[:, i, :],
                    xg[:, ci * 1024 + h * 512 : ci * 1024 + (h + 1) * 512],
                    start=(j == 0),
                    stop=(j == len(idxs) - 1),
                )

    with tile.TileContext(nc) as tc:
        with (
            tc.tile_pool(name="wp", bufs=1) as wp,
            tc.tile_pool(name="xp", bufs=2) as xp,
            tc.tile_pool(name="yp", bufs=2) as yp,
            tc.tile_pool(name="ep", bufs=ep_bufs) as ep,
            tc.tile_pool(name="qp", bufs=qp_bufs) as qp,
            tc.tile_pool(name="ps", bufs=2, space="PSUM") as psp,
        ):
            wb = wp.tile([128, nblk, 128], f16)
            nc.sync.dma_start(wb[:], wbd.ap()[:])
            tg = wp.tile([128, ntg], f32)
            nc.sync.dma_start(tg[:], trig.ap()[:])

            for sg in [s for _ in range(reps) for s in range(n_groups // 2)]:
                g0 = 2 * sg
                xt = xp.tile([128, 2, 8192], f16)
                for gg in range(2):
                    for ls in range(4):
                        nc.sync.dma_start(
                            xt[:, gg, ls * 2048 : (ls + 1) * 2048],
                            xin.ap()[g0 + gg][:, ls * 2048 : (ls + 1) * 2048],
                        )
                yt = yp.tile([128, 2, 8192], f16)

                # PE + pair-granular ACT eviction, per (pair, group)
                w = {}
                for ca, cb in ((0, 2), (1, 3), (4, 6), (5, 7)):
                    e = ep.tile([128, 2, 2048], f16, tag="e", name=f"w{ca}{cb}")
                    for gg in range(2):
                        p = psp.tile([128, 2048], f32, tag="ps")
                        mm_chunk(p[:, 0:1024], wb, ca, xt[:, gg, :])
                        mm_chunk(p[:, 1024:2048], wb, cb, xt[:, gg, :])
                        if stage_n >= 1:
                            nc.scalar.copy(e[:, gg, :], p[:])
                    w[ca] = e[:, :, 0:1024]
                    w[cb] = e[:, :, 1024:2048]
                if stage_n < 2:
                    continue

                # DVE stage 8 (2048-wide, strided): q'A = coefA*wB + wA
                q = {}
                for j, (A, B, _) in enumerate(pairs):
                    qA = qp.tile([128, 2, 1024], f16, tag="q", name=f"q{A}")
                    nc.vector.scalar_tensor_tensor(
                        qA[:], w[B], tg[:, 8 + 2 * j : 9 + 2 * j], w[A],
                        mult, add,
                    )
                    qB = qp.tile([128, 2, 1024], f16, tag="q", name=f"q{B}")
                    nc.vector.scalar_tensor_tensor(
                        qB[:], w[A], tg[:, 9 + 2 * j : 10 + 2 * j], w[B],
                        mult, add,
                    )
                    q[A], q[B] = qA[:], qB[:]
                for c in a_set:
                    q[c] = w[c]

                def store(gg, sl):
                    nc.gpsimd.dma_start(
                        yout.ap()[g0 + gg][:, sl : sl + 2048],
                        yt[:, gg, sl : sl + 2048],
                    )

                # DVE stage 9 (2048-wide): lo slots first, then hi
                for cg in range(4):
                    nc.vector.scalar_tensor_tensor(
                        yt[:, :, cg * 1024 : (cg + 1) * 1024],
                        q[cg + 4], tg[:, 4 + cg : 5 + cg], q[cg],
                        mult, add,
                    )
                    if stage_n >= 3 and cg % 2 == 1:
                        for gg in range(2):
                            store(gg, (cg - 1) * 1024)
                for cg in range(4):
                    nc.vector.scalar_tensor_tensor(
                        yt[:, :, (cg + 4) * 1024 : (cg + 5) * 1024],
                        q[cg], tg[:, cg : cg + 1], q[cg + 4],
                        mult, add,
                    )
                    if stage_n >= 3 and cg % 2 == 1:
                        for gg in range(2):
                            store(gg, (cg + 3) * 1024)
    _split_multi_waits(nc)
    return nc


_CACHE = {}

# selected configuration (see ab.py benchmarks)
CONFIG_SPLIT = "4B"
CONFIG_BUILDER = "sg2"  # sg2 == the default build_bass


def _get_nc(n_groups=N_GROUPS):
    if n_groups not in _CACHE:
        if CONFIG_BUILDER == "sg2":
            _CACHE[n_groups] = build_bass(n_groups, split=CONFIG_SPLIT)
        else:
            _CACHE[n_groups] = build_bass_std(n_groups, split=CONFIG_SPLIT)
    return _CACHE[n_groups]


def make_in_maps(x, angles, split=None):
    """Pack full inputs into per-core in_maps (list of dicts)."""
    x = np.asarray(x, dtype=np.float32)
    angles = np.asarray(angles, dtype=np.float32)
    wb, trig = _host_tables(angles, split or CONFIG_SPLIT)
    flat = x.reshape(-1, DIM).astype(np.float16)
    in_maps = []
    for k in range(N_CORES):
        shard = flat[k * ROWS_PER_CORE : (k + 1) * ROWS_PER_CORE]
        in_maps.append({"xin": _pack_x(shard), "wb": wb, "trig": trig})
    return in_maps


def kernel(x, angles):
    from concourse.bass_utils import run_bass_kernel_spmd

    x = np.asarray(x)
    orig_shape = x.shape
    in_maps = make_in_maps(x, angles)
    nc = _get_nc()
    res = run_bass_kernel_spmd(nc, in_maps, core_ids=list(range(N_CORES)))
    parts = [_unpack_y(res.results[k]["yout"]) for k in range(N_CORES)]
    out = np.concatenate(parts, axis=0).reshape(orig_shape)
    return out.astype(np.float32)
